# revision 9
# baseline (speedup 1.0000x reference)
"""Trainium2 Bass kernel for nn_BlockDrop (Swin-style transformer block).

Reference math (per batch image):
  h = LN1(x); 16x16 windows of 256 tokens; 16-head attention (d=64) with
  separate Q/K/V/O linears; x += attn; h2 = LN2(x); x += W2@gelu(W1@h2).

Sharding: pure data parallel — batch image b -> core b (16 windows each,
no cross-core communication). Host performs window reordering,
transposition (feature-major) and weight folding; the NEFF does the rest.

In-kernel: activations feature-major [C, T]; bf16 matmuls, fp32 PSUM
accumulation, fp32 residual stream. LayerNorm stats via ones-matmuls;
rsqrt as exp(-0.5*ln(var+eps)) so pass A needs one ACT table set. The
LN mean offset and all biases are folded in as rank-1 K=1 matmuls.
Softmax: scores^T layout, no max-subtraction (inputs bounded), ones
column appended to V yields the softmax denominators inside the o-matmul;
1/s rows are broadcast via selector matmuls.

SBUF: one NEFF, three passes (attn+LN2 | W1+gelu | W2+residual) with
DRAM intermediates; weight slots and most activation slots are
tag-shared across passes.
"""
import numpy as np
import ml_dtypes

import concourse.bass as bass
import concourse.mybir as mybir
import concourse.tile as tile
from concourse.bass_utils import run_bass_kernel_spmd

f32 = mybir.dt.float32
f32r = mybir.dt.float32r
bf16 = mybir.dt.bfloat16
AF = mybir.ActivationFunctionType

DIM = 1024
HEADS = 16
HDIM = 64
HID = 4096
SCALE = HDIM ** -0.5
EPS = 1e-5
T = 4096          # tokens per core
TT = 512          # tokens per T-tile (2 windows)
NC = 8            # C chunks
NH = 32           # HID chunks
WS2 = 256         # tokens per window


def _split_multi_waits(nc):
    """This walrus rejects >1 sync-wait per instruction. Move extra waits
    onto same-engine NoOps inserted just before (engine queues are FIFO,
    so blocking the queue on each sem in turn is equivalent)."""
    n_split = 0
    for fn in nc.m.functions:
        for blk in fn.blocks:
            insts = blk.instructions
            new = []
            for inst in insts:
                si = inst.sync_info
                waits = list(si.on_wait) if si is not None else []
                if len(waits) > 1:
                    for w in waits[:-1]:
                        n_split += 1
                        new.append(mybir.InstNoOp(
                            name=f"{inst.name}-ws{n_split}",
                            engine=inst.engine, ins=[], outs=[],
                            sync_info=mybir.SyncInfo(on_wait=[w], on_update=[]),
                        ))
                    inst.sync_info = mybir.SyncInfo(
                        on_wait=[waits[-1]], on_update=list(si.on_update))
                new.append(inst)
            if len(new) != len(insts):
                blk.instructions[:] = new
    return n_split


def build_nc(NT=8, use_f32r=False, xin_bufs=2):
    nc = bass.Bass()

    xT_e = nc.declare_dram_parameter("xT", [DIM, T], f32, isOutput=False)
    wq_e = nc.declare_dram_parameter("wq", [DIM, DIM], bf16, isOutput=False)
    wk_e = nc.declare_dram_parameter("wk", [DIM, DIM], bf16, isOutput=False)
    wv_e = nc.declare_dram_parameter("wv", [DIM, DIM], bf16, isOutput=False)
    wo_e = nc.declare_dram_parameter("wo", [DIM, DIM], bf16, isOutput=False)
    w1_e = nc.declare_dram_parameter("w1", [DIM, HID], bf16, isOutput=False)
    w2_e = nc.declare_dram_parameter("w2", [HID, DIM], bf16, isOutput=False)
    bor_e = nc.declare_dram_parameter("bor", [1, DIM], bf16, isOutput=False)
    b2r_e = nc.declare_dram_parameter("b2r", [1, DIM], bf16, isOutput=False)
    bqk_e = nc.declare_dram_parameter("bqk", [128, 16], f32, isOutput=False)
    b1c_e = nc.declare_dram_parameter("b1c", [128, NH], f32, isOutput=False)
    sel_e = nc.declare_dram_parameter("sel", [128, 256], bf16, isOutput=False)
    yT_e = nc.declare_dram_parameter("yT", [DIM, T], f32, isOutput=True)

    rd = nc.dram_tensor("rd", [DIM, T], f32)        # post-attn residual
    h2d = nc.dram_tensor("h2d", [DIM, T], bf16)     # LN2: r*rstd
    gd = nc.dram_tensor("gd", [HID, T], bf16)       # gelu(W1 h2 + b1)

    stat_dt = f32r if use_f32r else f32

    with tile.TileContext(nc) as tc:
        with (
            tc.tile_pool(name="wt", bufs=1) as wt,
            tc.tile_pool(name="cst", bufs=1) as cst,
            tc.tile_pool(name="act", bufs=1) as act,
            tc.tile_pool(name="psA", bufs=6, space="PSUM") as psA,
            tc.tile_pool(name="psS", bufs=2, space="PSUM") as psS,
        ):
            # ---- constants ----
            bor = cst.tile([1, DIM], bf16)
            b2r = cst.tile([1, DIM], bf16)
            bqk = cst.tile([128, 16], f32)
            b1c = cst.tile([128, NH], f32)
            sel = cst.tile([128, 256], bf16)
            for dst, srcp in ((bor, bor_e), (b2r, b2r_e),
                              (bqk, bqk_e), (b1c, b1c_e), (sel, sel_e)):
                nc.sync.dma_start(out=dst, in_=srcp[:])
            ones_s = cst.tile([128, 1], f32)     # LN sum lhsT
            ones_q = cst.tile([128, 1], bf16)    # LN sumsq lhsT
            ones_r = cst.tile([1, TT], bf16)     # bias-fold rhs
            ones_b = cst.tile([1, 128], bf16)    # K=1 broadcast lhsT
            eps_t = cst.tile([1, 1], f32)
            nc.vector.memset(ones_s, 1.0)
            nc.vector.memset(ones_q, 1.0)
            nc.vector.memset(ones_r, 1.0)
            nc.vector.memset(ones_b, 1.0)
            nc.vector.memset(eps_t, EPS)

            # ---- pass-A weights in the 32 shared weight slots ----
            wq_sb, wk_sb, wv_sb, wo_sb = [], [], [], []
            for g, (lst, src) in enumerate((
                    (wq_sb, wq_e), (wk_sb, wk_e), (wv_sb, wv_e), (wo_sb, wo_e))):
                for c in range(NC):
                    t_ = wt.tile([128, DIM], bf16, name=f"wA{g}_{c}", tag=f"wt{g * 8 + c}")
                    nc.sync.dma_start(out=t_, in_=src[c * 128:(c + 1) * 128, :])
                    lst.append(t_)

            def layernorm(src_tiles, dst_tiles, tag):
                """dst = (src - mean) * rstd (bf16), fully normalized."""
                ps_s = psS.tile([1, TT], f32, name=f"ps_s{tag}", tag="psS")
                ps_q = psS.tile([1, TT], f32, name=f"ps_q{tag}", tag="psS")
                for c in range(NC):
                    sq = act.tile([128, TT], bf16, name=f"sq{tag}{c}", tag="sq", bufs=2)
                    nc.scalar.activation(sq, src_tiles[c], AF.Square)
                    nc.tensor.matmul(ps_s, lhsT=ones_s.bitcast(stat_dt),
                                     rhs=src_tiles[c].bitcast(stat_dt),
                                     start=(c == 0), stop=(c == NC - 1))
                    nc.tensor.matmul(ps_q, lhsT=ones_q, rhs=sq,
                                     start=(c == 0), stop=(c == NC - 1))
                mean = act.tile([1, TT], bf16, name=f"mean{tag}", tag="r_mean", bufs=2)
                meanf = act.tile([1, TT], f32, name=f"meanf{tag}", tag="r_meanf", bufs=1)
                exq = act.tile([1, TT], f32, name=f"exq{tag}", tag="r_exq", bufs=2)
                nc.scalar.activation(mean, ps_s, AF.Copy, scale=1.0 / DIM)
                nc.scalar.activation(meanf, ps_s, AF.Copy, scale=1.0 / DIM)
                nc.scalar.activation(exq, ps_q, AF.Copy, scale=1.0 / DIM)
                m2 = act.tile([1, TT], f32, name=f"m2{tag}", tag="r_m2", bufs=1)
                nc.scalar.activation(m2, meanf, AF.Square)
                nc.vector.tensor_sub(exq, exq, m2)          # var (in place)
                lnv = act.tile([1, TT], f32, name=f"lnv{tag}", tag="r_lnv", bufs=1)
                nc.scalar.activation(lnv, exq, AF.Ln, bias=eps_t)
                rs_b = act.tile([1, TT], bf16, name=f"rsb{tag}", tag="r_rsb", bufs=2)
                nc.scalar.activation(rs_b, lnv, AF.Exp, scale=-0.5)
                ps_m = psA.tile([128, TT], f32, name=f"ps_m{tag}", tag="psA")
                nc.tensor.matmul(ps_m, lhsT=ones_b, rhs=mean, start=True, stop=True)
                ps_r = psA.tile([128, TT], f32, name=f"ps_r{tag}", tag="psA")
                nc.tensor.matmul(ps_r, lhsT=ones_b, rhs=rs_b, start=True, stop=True)
                for c in range(NC):
                    cen = act.tile([128, TT], f32, name=f"cen{tag}{c}", tag="cen", bufs=1)
                    nc.vector.tensor_sub(cen, src_tiles[c], ps_m)
                    nc.vector.tensor_mul(dst_tiles[c], cen, ps_r)

            # ======== PASS A0: LN1 stats for all tiles (pipelined) ========
            mean_all = cst.tile([1, T], bf16)
            rs1_all = cst.tile([1, T], bf16)
            for it in range(NT):
                t0 = it * TT
                xa = [act.tile([128, TT], f32, name=f"xa{c}", tag=f"xt{c}", bufs=xin_bufs)
                      for c in range(NC)]
                for c in range(NC):
                    nc.sync.dma_start(out=xa[c], in_=xT_e[c * 128:(c + 1) * 128, t0:t0 + TT])
                ps_s = psS.tile([1, TT], f32, name="ps_sA0", tag="psS")
                ps_q = psS.tile([1, TT], f32, name="ps_qA0", tag="psS")
                for c in range(NC):
                    sq = act.tile([128, TT], bf16, name=f"sqA0{c}", tag="sq", bufs=2)
                    nc.scalar.activation(sq, xa[c], AF.Square)
                    nc.tensor.matmul(ps_s, lhsT=ones_s.bitcast(stat_dt),
                                     rhs=xa[c].bitcast(stat_dt),
                                     start=(c == 0), stop=(c == NC - 1))
                    nc.tensor.matmul(ps_q, lhsT=ones_q, rhs=sq,
                                     start=(c == 0), stop=(c == NC - 1))
                meanf = act.tile([1, TT], f32, name="meanfA0", tag="r_meanf", bufs=1)
                exq = act.tile([1, TT], f32, name="exqA0", tag="r_exq", bufs=2)
                nc.scalar.activation(mean_all[0:1, t0:t0 + TT], ps_s, AF.Copy, scale=1.0 / DIM)
                nc.scalar.activation(meanf, ps_s, AF.Copy, scale=1.0 / DIM)
                nc.scalar.activation(exq, ps_q, AF.Copy, scale=1.0 / DIM)
                m2 = act.tile([1, TT], f32, name="m2A0", tag="r_m2", bufs=1)
                nc.scalar.activation(m2, meanf, AF.Square)
                nc.vector.tensor_sub(exq, exq, m2)
                lnv = act.tile([1, TT], f32, name="lnvA0", tag="r_lnv", bufs=1)
                nc.scalar.activation(lnv, exq, AF.Ln, bias=eps_t)
                nc.scalar.activation(rs1_all[0:1, t0:t0 + TT], lnv, AF.Exp, scale=-0.5)

            # =========================== PASS A ===========================
            for it in range(NT):
                t0 = it * TT
                xt = [act.tile([128, TT], f32, name=f"xt{c}", tag=f"xt{c}", bufs=xin_bufs)
                      for c in range(NC)]
                for c in range(NC):
                    nc.sync.dma_start(out=xt[c], in_=xT_e[c * 128:(c + 1) * 128, t0:t0 + TT])
                hb = [act.tile([128, TT], bf16, name=f"hb{c}", tag=f"hb{c}")
                      for c in range(NC)]
                ps_m1 = psA.tile([128, TT], f32, name="ps_m1", tag="psA")
                nc.tensor.matmul(ps_m1, lhsT=ones_b, rhs=mean_all[0:1, t0:t0 + TT],
                                 start=True, stop=True)
                ps_r1 = psA.tile([128, TT], f32, name="ps_r1", tag="psA")
                nc.tensor.matmul(ps_r1, lhsT=ones_b, rhs=rs1_all[0:1, t0:t0 + TT],
                                 start=True, stop=True)
                for c in range(NC):
                    cen = act.tile([128, TT], f32, name=f"cenL1{c}", tag="cen", bufs=1)
                    nc.vector.tensor_sub(cen, xt[c], ps_m1)
                    nc.vector.tensor_mul(hb[c], cen, ps_r1)

                # ---- QKV ----
                q_sb = [act.tile([128, TT], bf16, name=f"q{c}", tag=f"q{c}") for c in range(NC)]
                k_sb = [act.tile([128, TT], bf16, name=f"k{c}", tag=f"k{c}") for c in range(NC)]
                for co in range(NC):
                    ps = psA.tile([128, TT], f32, name="ps_q", tag="psA")
                    for c in range(NC):
                        nc.tensor.matmul(ps, lhsT=wq_sb[c][:, co * 128:(co + 1) * 128],
                                         rhs=hb[c], start=(c == 0), stop=(c == NC - 1))
                    nc.any.tensor_scalar_add(q_sb[co], ps, bqk[:, co:co + 1])
                    ps = psA.tile([128, TT], f32, name="ps_k", tag="psA")
                    for c in range(NC):
                        nc.tensor.matmul(ps, lhsT=wk_sb[c][:, co * 128:(co + 1) * 128],
                                         rhs=hb[c], start=(c == 0), stop=(c == NC - 1))
                    nc.any.tensor_scalar_add(k_sb[co], ps, bqk[:, 8 + co:8 + co + 1])
                v_sb = [act.tile([128, HEADS, 65], bf16, name=f"v{tc_}", tag=f"v{tc_}")
                        for tc_ in range(4)]
                for tc_ in range(4):
                    for nh in range(2):
                        ps = psA.tile([128, TT], f32, name="ps_v", tag="psA")
                        for c in range(NC):
                            nc.tensor.matmul(ps, lhsT=hb[c][:, tc_ * 128:(tc_ + 1) * 128],
                                             rhs=wv_sb[c][:, nh * 512:(nh + 1) * 512],
                                             start=(c == 0), stop=(c == NC - 1))
                        nc.vector.tensor_copy(
                            v_sb[tc_][:, nh * 8:(nh + 1) * 8, 0:64],
                            ps.rearrange("p (h d) -> p h d", d=64))
                    nc.vector.memset(v_sb[tc_][:, :, 64:65], 1.0)

                # ---- attention ----
                sc = [act.tile([128, TT], bf16, name=f"sc{g}", tag=f"sc{g}") for g in range(4)]
                for g in range(4):
                    nc.vector.memset(sc[g], 1.0)
                oT = [act.tile([128, TT], bf16, name=f"oT{c}", tag=f"oT{c}") for c in range(NC)]
                for w in range(2):
                    ws = w * WS2
                    for h in range(HEADS):
                        ch, hh = h // 2, 64 * (h % 2)
                        ps_s = psA.tile([128, TT], f32, name="ps_sT", tag="psA")
                        nc.tensor.matmul(ps_s[:, 0:WS2],
                                         lhsT=k_sb[ch][hh:hh + 64, ws:ws + 128],
                                         rhs=q_sb[ch][hh:hh + 64, ws:ws + WS2],
                                         start=True, stop=False)
                        nc.tensor.matmul(ps_s[:, WS2:TT],
                                         lhsT=k_sb[ch][hh:hh + 64, ws + 128:ws + WS2],
                                         rhs=q_sb[ch][hh:hh + 64, ws:ws + WS2],
                                         start=False, stop=True)
                        e_sb = act.tile([128, TT], bf16, name="e_sb", tag="e", bufs=2)
                        nc.scalar.activation(e_sb, ps_s, AF.Exp)
                        ps_o = psA.tile([65, WS2], f32, name="ps_o", tag="psA")
                        nc.tensor.matmul(ps_o, lhsT=v_sb[2 * w][:, h, :],
                                         rhs=e_sb[:, 0:WS2], start=True, stop=False)
                        nc.tensor.matmul(ps_o, lhsT=v_sb[2 * w + 1][:, h, :],
                                         rhs=e_sb[:, WS2:TT], start=False, stop=True)
                        nc.any.tensor_copy(
                            sc[h // 4][32 * (h % 4):32 * (h % 4) + 1, ws:ws + WS2],
                            ps_o[64:65, :])
                        nc.any.tensor_copy(oT[ch][hh:hh + 64, ws:ws + WS2], ps_o[0:64, :])

                # ---- normalize (in place) + Wo + residual ----
                with nc.allow_low_precision(reason="1/s as bf16 matmul operand"):
                    for g in range(4):
                        nc.vector.reciprocal(sc[g], sc[g])
                for j in range(NC):
                    ps_b = psA.tile([128, TT], f32, name="ps_rsb", tag="psA")
                    nc.tensor.matmul(ps_b, lhsT=sel[:, 128 * (j % 2):128 * (j % 2) + 128],
                                     rhs=sc[j // 2], start=True, stop=True)
                    nc.vector.tensor_mul(oT[j], oT[j], ps_b)
                r_sb = [act.tile([128, TT], f32, name=f"r{c}", tag=f"r{c}") for c in range(NC)]
                for co in range(NC):
                    ps = psA.tile([128, TT], f32, name="ps_wo", tag="psA")
                    for c in range(NC):
                        nc.tensor.matmul(ps, lhsT=wo_sb[c][:, co * 128:(co + 1) * 128],
                                         rhs=oT[c], start=(c == 0), stop=False)
                    nc.tensor.matmul(ps, lhsT=bor[0:1, co * 128:(co + 1) * 128],
                                     rhs=ones_r, start=False, stop=True)
                    nc.vector.tensor_add(r_sb[co], ps, xt[co])
                    nc.sync.dma_start(out=rd[co * 128:(co + 1) * 128, t0:t0 + TT], in_=r_sb[co])

                # ---- LN2 ----
                h2 = [act.tile([128, TT], bf16, name=f"h2_{c}", tag=f"h2_{c}", bufs=1)
                      for c in range(NC)]
                layernorm(r_sb, h2, "L2")
                for c in range(NC):
                    nc.sync.dma_start(out=h2d[c * 128:(c + 1) * 128, t0:t0 + TT], in_=h2[c])

            # =========================== PASS B1 (W1 + gelu) ==============
            w1_sb = []
            for i in range(NC * 4):
                c, qd = i // 4, i % 4
                t_ = wt.tile([128, DIM], bf16, name=f"w1_{i}", tag=f"wt{i}")
                nc.sync.dma_start(out=t_, in_=w1_e[c * 128:(c + 1) * 128,
                                                   qd * DIM:(qd + 1) * DIM])
                w1_sb.append(t_)
            for it in range(NT):
                t0 = it * TT
                h2b = [act.tile([128, TT], bf16, name=f"h2b{c}", tag=f"h2_{c}", bufs=1)
                       for c in range(NC)]
                for c in range(NC):
                    nc.sync.dma_start(out=h2b[c], in_=h2d[c * 128:(c + 1) * 128, t0:t0 + TT])
                for hj in range(NH):
                    qd, sub = hj // 8, hj % 8
                    ps = psA.tile([128, TT], f32, name="ps_w1", tag="psA")
                    for c in range(NC):
                        nc.tensor.matmul(ps, lhsT=w1_sb[c * 4 + qd][:, sub * 128:(sub + 1) * 128],
                                         rhs=h2b[c], start=(c == 0), stop=(c == NC - 1))
                    g_sb = act.tile([128, TT], bf16, name="g_sb", tag="sq", bufs=2)
                    nc.scalar.activation(g_sb, ps, AF.Gelu, bias=b1c[:, hj:hj + 1])
                    nc.sync.dma_start(out=gd[hj * 128:(hj + 1) * 128, t0:t0 + TT], in_=g_sb)

            # =========================== PASS B2 (W2 + residual) ==========
            w2_sb = []
            for i in range(NH):
                t_ = wt.tile([128, DIM], bf16, name=f"w2_{i}", tag=f"wt{i}")
                nc.sync.dma_start(out=t_, in_=w2_e[i * 128:(i + 1) * 128, :])
                w2_sb.append(t_)
            GB_TAGS = [f"hb{i}" for i in range(8)] + [f"q{i}" for i in range(8)] + \
                      [f"k{i}" for i in range(8)] + [f"oT{i}" for i in range(8)]
            for it in range(NT):
                t0 = it * TT
                gb = [act.tile([128, TT], bf16, name=f"gb{hc}", tag=GB_TAGS[hc])
                      for hc in range(NH)]
                for hc in range(NH):
                    nc.sync.dma_start(out=gb[hc], in_=gd[hc * 128:(hc + 1) * 128, t0:t0 + TT])
                rb = [act.tile([128, TT], f32, name=f"rb{c}", tag=f"r{c}") for c in range(NC)]
                for c in range(NC):
                    nc.sync.dma_start(out=rb[c], in_=rd[c * 128:(c + 1) * 128, t0:t0 + TT])
                for co in range(NC):
                    ps = psA.tile([128, TT], f32, name="ps_w2", tag="psA")
                    for hc in range(NH):
                        nc.tensor.matmul(ps, lhsT=w2_sb[hc][:, co * 128:(co + 1) * 128],
                                         rhs=gb[hc], start=(hc == 0), stop=False)
                    nc.tensor.matmul(ps, lhsT=b2r[0:1, co * 128:(co + 1) * 128],
                                     rhs=ones_r, start=False, stop=True)
                    y_sb = act.tile([128, TT], f32, name="y_sb", tag=f"xt{co}", bufs=xin_bufs)
                    nc.vector.tensor_add(y_sb, ps, rb[co])
                    nc.sync.dma_start(out=yT_e[co * 128:(co + 1) * 128, t0:t0 + TT], in_=y_sb)

    _split_multi_waits(nc)
    return nc


# ---------------------------------------------------------------------------
# Host side
# ---------------------------------------------------------------------------
_CACHE = {}


def _bf(a):
    return np.ascontiguousarray(a).astype(ml_dtypes.bfloat16)


def prep_consts(g1, beta1, Wq, bq, Wk, bk, Wv, bv, Wo, bo, g2, beta2,
                W1, b1m, W2, b2m):
    Wq_e = (g1[:, None] * Wq) * SCALE
    bq_e = (beta1 @ Wq + bq) * SCALE
    Wk_e = g1[:, None] * Wk
    bk_e = beta1 @ Wk + bk
    Wv_e = g1[:, None] * Wv
    bv_e = beta1 @ Wv + bv
    bo_e = bv_e @ Wo + bo
    W1_e = g2[:, None] * W1
    b1_e = beta2 @ W1 + b1m
    # cols 0-7: bq chunks; cols 8-15: bk chunks
    bqk = np.concatenate([bq_e.reshape(8, 128).T, bk_e.reshape(8, 128).T], axis=1)
    sel = np.zeros((128, 256), np.float32)
    sel[0, 0:64] = 1.0       # even chunk: heads at rows 0 / 32
    sel[32, 64:128] = 1.0
    sel[64, 128 + 0:128 + 64] = 1.0   # odd chunk: rows 64 / 96
    sel[96, 128 + 64:128 + 128] = 1.0
    return {
        "wq": _bf(Wq_e), "wk": _bf(Wk_e), "wv": _bf(Wv_e), "wo": _bf(Wo),
        "w1": _bf(W1_e), "w2": _bf(W2),
        "bor": _bf(bo_e)[None, :], "b2r": _bf(b2m)[None, :],
        "bqk": np.ascontiguousarray(bqk.astype(np.float32)),
        "b1c": np.ascontiguousarray(b1_e.reshape(NH, 128).T.astype(np.float32)),
        "sel": _bf(sel),
    }


def window_order(x_b):
    # [4096, C] row-major spatial -> window-contiguous [4096, C]
    C = x_b.shape[-1]
    t = x_b.reshape(4, 16, 4, 16, C).transpose(0, 2, 1, 3, 4)
    return t.reshape(4096, C)


def window_unorder(y_b):
    C = y_b.shape[-1]
    t = y_b.reshape(4, 4, 16, 16, C).transpose(0, 2, 1, 3, 4)
    return t.reshape(4096, C)


def kernel(x, g1, beta1, Wq, bq, Wk, bk, Wv, bv, Wo, bo, g2, beta2,
           W1, b1m, W2, b2m, window_size, spatial_h, spatial_w):
    x = np.asarray(x, np.float32)
    args = [np.asarray(a, np.float32) for a in
            (g1, beta1, Wq, bq, Wk, bk, Wv, bv, Wo, bo, g2, beta2, W1, b1m, W2, b2m)]
    consts = prep_consts(*args)

    if "nc" not in _CACHE:
        _CACHE["nc"] = build_nc(NT=8)
    nc = _CACHE["nc"]

    B = x.shape[0]
    in_maps = []
    for c in range(B):
        xw = window_order(x[c])                       # [4096, C]
        m = {"xT": np.ascontiguousarray(xw.T)}        # [C, 4096] f32
        m.update(consts)
        in_maps.append(m)
    res = run_bass_kernel_spmd(nc, in_maps, core_ids=list(range(B)))
    out = np.empty_like(x)
    for c in range(B):
        yT = res.results[c]["yT"]                     # [C, 4096]
        out[c] = window_unorder(np.ascontiguousarray(yT.T))
    return out


# revision 15
# speedup vs baseline: 1.1648x; 1.1648x over previous
"""Trainium2 Bass kernel for nn_BlockDrop (Swin-style transformer block).

Reference math (per batch image):
  h = LN1(x); 16x16 windows of 256 tokens; 16-head attention (d=64) with
  separate Q/K/V/O linears; x += attn; h2 = LN2(x); x += W2@gelu(W1@h2).

Sharding: pure data parallel — batch image b -> core b (16 windows each,
no cross-core communication). Host performs window reordering,
transposition (feature-major) and weight folding; the NEFF does the rest.

In-kernel: activations feature-major [C, T]; bf16 matmuls, fp32 PSUM
accumulation, fp32 residual stream. LayerNorm stats via ones-matmuls;
rsqrt as exp(-0.5*ln(var+eps)) so pass A needs one ACT table set. The
LN mean offset and all biases are folded in as rank-1 K=1 matmuls.
Softmax: scores^T layout, no max-subtraction (inputs bounded), ones
column appended to V yields the softmax denominators inside the o-matmul;
1/s rows are broadcast via selector matmuls.

SBUF: one NEFF, three passes (attn+LN2 | W1+gelu | W2+residual) with
DRAM intermediates; weight slots and most activation slots are
tag-shared across passes.
"""
import numpy as np
import ml_dtypes

import concourse.bass as bass
import concourse.mybir as mybir
import concourse.tile as tile
from concourse.bass_utils import run_bass_kernel_spmd

f32 = mybir.dt.float32
f32r = mybir.dt.float32r
bf16 = mybir.dt.bfloat16
AF = mybir.ActivationFunctionType

DIM = 1024
HEADS = 16
HDIM = 64
HID = 4096
SCALE = HDIM ** -0.5
EPS = 1e-5
T = 4096          # tokens per core
TT = 512          # tokens per T-tile (2 windows)
NC = 8            # C chunks
NH = 32           # HID chunks
WS2 = 256         # tokens per window


def _split_multi_waits(nc):
    """This walrus rejects >1 sync-wait per instruction. Move extra waits
    onto same-engine NoOps inserted just before (engine queues are FIFO,
    so blocking the queue on each sem in turn is equivalent)."""
    n_split = 0
    for fn in nc.m.functions:
        for blk in fn.blocks:
            insts = blk.instructions
            new = []
            for inst in insts:
                si = inst.sync_info
                waits = list(si.on_wait) if si is not None else []
                if len(waits) > 1:
                    for w in waits[:-1]:
                        n_split += 1
                        new.append(mybir.InstNoOp(
                            name=f"{inst.name}-ws{n_split}",
                            engine=inst.engine, ins=[], outs=[],
                            sync_info=mybir.SyncInfo(on_wait=[w], on_update=[]),
                        ))
                    inst.sync_info = mybir.SyncInfo(
                        on_wait=[waits[-1]], on_update=list(si.on_update))
                new.append(inst)
            if len(new) != len(insts):
                blk.instructions[:] = new
    return n_split


def build_nc(NT=8, use_f32r=False, xin_bufs=1):
    nc = bass.Bass()

    xT_e = nc.declare_dram_parameter("xT", [DIM, T], f32, isOutput=False)
    wq_e = nc.declare_dram_parameter("wq", [DIM, DIM], bf16, isOutput=False)
    wk_e = nc.declare_dram_parameter("wk", [DIM, DIM], bf16, isOutput=False)
    wv_e = nc.declare_dram_parameter("wv", [DIM, DIM], bf16, isOutput=False)
    wo_e = nc.declare_dram_parameter("wo", [DIM, DIM], bf16, isOutput=False)
    w1_e = nc.declare_dram_parameter("w1", [DIM, HID], bf16, isOutput=False)
    w2_e = nc.declare_dram_parameter("w2", [HID, DIM], bf16, isOutput=False)
    bor_e = nc.declare_dram_parameter("bor", [1, DIM], bf16, isOutput=False)
    b2r_e = nc.declare_dram_parameter("b2r", [1, DIM], bf16, isOutput=False)
    bqk_e = nc.declare_dram_parameter("bqk", [128, 16], f32, isOutput=False)
    b1c_e = nc.declare_dram_parameter("b1c", [128, NH], f32, isOutput=False)
    sel_e = nc.declare_dram_parameter("sel", [128, 256], bf16, isOutput=False)
    yT_e = nc.declare_dram_parameter("yT", [DIM, T], f32, isOutput=True)

    rd = nc.dram_tensor("rd", [DIM, T], f32)        # post-attn residual
    m2d = nc.dram_tensor("m2d", [1, T], bf16)       # LN2 mean row
    r2d = nc.dram_tensor("r2d", [1, T], bf16)       # LN2 rstd row
    gd = nc.dram_tensor("gd", [HID, T], bf16)       # gelu(W1 h2 + b1)

    stat_dt = f32r if use_f32r else f32

    with tile.TileContext(nc) as tc:
        with (
            tc.tile_pool(name="wt", bufs=1) as wt,
            tc.tile_pool(name="cst", bufs=1) as cst,
            tc.tile_pool(name="act", bufs=1) as act,
            tc.tile_pool(name="psA", bufs=8, space="PSUM") as psA,
        ):
            # ---- constants ----
            bor = cst.tile([1, DIM], bf16)
            b2r = cst.tile([1, DIM], bf16)
            bqk = cst.tile([128, 16], f32)
            b1c = cst.tile([128, NH], f32)
            sel = cst.tile([128, 256], bf16)
            for dst, srcp in ((bor, bor_e), (b2r, b2r_e),
                              (bqk, bqk_e), (b1c, b1c_e), (sel, sel_e)):
                nc.sync.dma_start(out=dst, in_=srcp[:])
            ones_s = cst.tile([128, 1], f32)     # LN sum lhsT
            ones_q = cst.tile([128, 1], bf16)    # LN sumsq lhsT
            ones_r = cst.tile([1, TT], bf16)     # bias-fold rhs
            ones_b = cst.tile([1, 128], bf16)    # K=1 broadcast lhsT
            eps_t = cst.tile([1, 1], f32)
            nc.vector.memset(ones_s, 1.0)
            nc.vector.memset(ones_q, 1.0)
            nc.vector.memset(ones_r, 1.0)
            nc.vector.memset(ones_b, 1.0)
            nc.vector.memset(eps_t, EPS)

            # ---- pass-A weights in the 32 shared weight slots ----
            wq_sb, wk_sb, wv_sb, wo_sb = [], [], [], []
            for g, (lst, src) in enumerate((
                    (wq_sb, wq_e), (wk_sb, wk_e), (wv_sb, wv_e), (wo_sb, wo_e))):
                for c in range(NC):
                    t_ = wt.tile([128, DIM], bf16, name=f"wA{g}_{c}", tag=f"wt{g * 8 + c}")
                    nc.sync.dma_start(out=t_, in_=src[c * 128:(c + 1) * 128, :])
                    lst.append(t_)

            def ln_stats(src_tiles, mean_dst, rs_dst, tag):
                """mean/rstd (bf16 [1,TT] rows) of feature-major src tiles."""
                ps_s = psA.tile([1, TT], f32, name=f"ps_s{tag}", tag="psA")
                ps_q = psA.tile([1, TT], f32, name=f"ps_q{tag}", tag="psA")
                for c in range(NC):
                    sq = act.tile([128, TT], bf16, name=f"sq{tag}{c}", tag="sq", bufs=2)
                    nc.scalar.activation(sq, src_tiles[c], AF.Square)
                    nc.tensor.matmul(ps_s, lhsT=ones_s.bitcast(stat_dt),
                                     rhs=src_tiles[c].bitcast(stat_dt),
                                     start=(c == 0), stop=(c == NC - 1))
                    nc.tensor.matmul(ps_q, lhsT=ones_q, rhs=sq,
                                     start=(c == 0), stop=(c == NC - 1))
                meanf = act.tile([1, TT], f32, name=f"meanf{tag}", tag="r_meanf", bufs=1)
                exq = act.tile([1, TT], f32, name=f"exq{tag}", tag="r_exq", bufs=2)
                nc.scalar.activation(mean_dst, ps_s, AF.Copy, scale=1.0 / DIM)
                nc.scalar.activation(meanf, ps_s, AF.Copy, scale=1.0 / DIM)
                nc.scalar.activation(exq, ps_q, AF.Copy, scale=1.0 / DIM)
                m2 = act.tile([1, TT], f32, name=f"m2{tag}", tag="r_m2", bufs=1)
                nc.scalar.activation(m2, meanf, AF.Square)
                nc.vector.tensor_sub(exq, exq, m2)          # var (in place)
                lnv = act.tile([1, TT], f32, name=f"lnv{tag}", tag="r_lnv", bufs=1)
                nc.scalar.activation(lnv, exq, AF.Ln, bias=eps_t)
                nc.scalar.activation(rs_dst, lnv, AF.Exp, scale=-0.5)

            def ln_apply(src_tiles, mean_row, rs_row, dst_tiles, tag):
                ps_m = psA.tile([128, TT], f32, name=f"ps_m{tag}", tag="psA")
                nc.tensor.matmul(ps_m, lhsT=ones_b, rhs=mean_row, start=True, stop=True)
                ps_r = psA.tile([128, TT], f32, name=f"ps_r{tag}", tag="psA")
                nc.tensor.matmul(ps_r, lhsT=ones_b, rhs=rs_row, start=True, stop=True)
                for c in range(NC):
                    cen = act.tile([128, TT], f32, name=f"cen{tag}{c}", tag="cen", bufs=1)
                    nc.vector.tensor_sub(cen, src_tiles[c], ps_m)
                    nc.vector.tensor_mul(dst_tiles[c], cen, ps_r)

            # ======== PASS A0: LN1 stats for all tiles (pipelined) ========
            mean_all = cst.tile([1, T], bf16)
            rs1_all = cst.tile([1, T], bf16)
            for it in range(NT):
                t0 = it * TT
                xa = [act.tile([128, TT], f32, name=f"xa{c}", tag=f"xt{c}", bufs=xin_bufs)
                      for c in range(NC)]
                for c in range(NC):
                    nc.sync.dma_start(out=xa[c], in_=xT_e[c * 128:(c + 1) * 128, t0:t0 + TT])
                ln_stats(xa, mean_all[0:1, t0:t0 + TT], rs1_all[0:1, t0:t0 + TT], "A0")

            # =========================== PASS A ===========================
            for it in range(NT):
                t0 = it * TT
                xt = [act.tile([128, TT], f32, name=f"xt{c}", tag=f"xt{c}", bufs=xin_bufs)
                      for c in range(NC)]
                for c in range(NC):
                    nc.sync.dma_start(out=xt[c], in_=xT_e[c * 128:(c + 1) * 128, t0:t0 + TT])
                hb = [act.tile([128, TT], bf16, name=f"hb{c}", tag=f"hb{c}")
                      for c in range(NC)]
                ln_apply(xt, mean_all[0:1, t0:t0 + TT], rs1_all[0:1, t0:t0 + TT], hb, "L1")

                # ---- QKV ----
                q_sb = [act.tile([128, TT], bf16, name=f"q{c}", tag=f"q{c}", bufs=2) for c in range(NC)]
                k_sb = [act.tile([128, TT], bf16, name=f"k{c}", tag=f"k{c}", bufs=2) for c in range(NC)]
                for co in range(NC):
                    ps = psA.tile([128, TT], f32, name="ps_q", tag="psA")
                    for c in range(NC):
                        nc.tensor.matmul(ps, lhsT=wq_sb[c][:, co * 128:(co + 1) * 128],
                                         rhs=hb[c], start=(c == 0), stop=(c == NC - 1))
                    nc.any.tensor_scalar_add(q_sb[co], ps, bqk[:, co:co + 1])
                    ps = psA.tile([128, TT], f32, name="ps_k", tag="psA")
                    for c in range(NC):
                        nc.tensor.matmul(ps, lhsT=wk_sb[c][:, co * 128:(co + 1) * 128],
                                         rhs=hb[c], start=(c == 0), stop=(c == NC - 1))
                    nc.any.tensor_scalar_add(k_sb[co], ps, bqk[:, 8 + co:8 + co + 1])
                v_sb = [act.tile([128, HEADS, 65], bf16, name=f"v{tc_}", tag=f"v{tc_}")
                        for tc_ in range(4)]
                for tc_ in range(4):
                    for nh in range(2):
                        ps = psA.tile([128, TT], f32, name="ps_v", tag="psA")
                        for c in range(NC):
                            nc.tensor.matmul(ps, lhsT=hb[c][:, tc_ * 128:(tc_ + 1) * 128],
                                             rhs=wv_sb[c][:, nh * 512:(nh + 1) * 512],
                                             start=(c == 0), stop=(c == NC - 1))
                        nc.vector.tensor_copy(
                            v_sb[tc_][:, nh * 8:(nh + 1) * 8, 0:64],
                            ps.rearrange("p (h d) -> p h d", d=64))
                    nc.vector.memset(v_sb[tc_][:, :, 64:65], 1.0)

                # ---- attention ----
                sc = [act.tile([128, TT], bf16, name=f"sc{g}", tag=f"sc{g}", bufs=1) for g in range(4)]
                for g in range(4):
                    nc.vector.memset(sc[g], 1.0)
                oT = [act.tile([128, TT], bf16, name=f"oT{c}", tag=f"oT{c}") for c in range(NC)]
                for w in range(2):
                    ws = w * WS2
                    for h0 in range(0, HEADS, 3):
                        grp = range(h0, min(h0 + 3, HEADS))
                        ps_s_g, e_g, ps_o_g = {}, {}, {}
                        for h in grp:
                            ch, hh = h // 2, 64 * (h % 2)
                            ps_s = psA.tile([128, TT], f32, name="ps_sT", tag="psA")
                            nc.tensor.matmul(ps_s[:, 0:WS2],
                                             lhsT=k_sb[ch][hh:hh + 64, ws:ws + 128],
                                             rhs=q_sb[ch][hh:hh + 64, ws:ws + WS2],
                                             start=True, stop=False)
                            nc.tensor.matmul(ps_s[:, WS2:TT],
                                             lhsT=k_sb[ch][hh:hh + 64, ws + 128:ws + WS2],
                                             rhs=q_sb[ch][hh:hh + 64, ws:ws + WS2],
                                             start=False, stop=True)
                            ps_s_g[h] = ps_s
                        for h in grp:
                            e_sb = act.tile([128, TT], bf16, name="e_sb", tag="e", bufs=3)
                            nc.scalar.activation(e_sb, ps_s_g[h], AF.Exp)
                            e_g[h] = e_sb
                        for h in grp:
                            ps_o = psA.tile([65, WS2], f32, name="ps_o", tag="psA")
                            nc.tensor.matmul(ps_o, lhsT=v_sb[2 * w][:, h, :],
                                             rhs=e_g[h][:, 0:WS2], start=True, stop=False)
                            nc.tensor.matmul(ps_o, lhsT=v_sb[2 * w + 1][:, h, :],
                                             rhs=e_g[h][:, WS2:TT], start=False, stop=True)
                            ps_o_g[h] = ps_o
                        for h in grp:
                            ch, hh = h // 2, 64 * (h % 2)
                            nc.vector.tensor_copy(
                                sc[h // 4][32 * (h % 4):32 * (h % 4) + 1, ws:ws + WS2],
                                ps_o_g[h][64:65, :])
                            nc.any.tensor_copy(oT[ch][hh:hh + 64, ws:ws + WS2],
                                               ps_o_g[h][0:64, :])

                # ---- normalize (in place) + Wo + residual ----
                with nc.allow_low_precision(reason="1/s as bf16 matmul operand"):
                    for g in range(2):
                        nc.scalar.activation(sc[g], sc[g], AF.Ln)
                        nc.scalar.activation(sc[g], sc[g], AF.Exp, scale=-1.0)
                    for g in range(2, 4):
                        nc.vector.reciprocal(sc[g], sc[g])
                for j in range(NC):
                    ps_b = psA.tile([128, TT], f32, name="ps_rsb", tag="psA")
                    nc.tensor.matmul(ps_b, lhsT=sel[:, 128 * (j % 2):128 * (j % 2) + 128],
                                     rhs=sc[j // 2], start=True, stop=True)
                    nc.vector.tensor_mul(oT[j], oT[j], ps_b)
                r_sb = [act.tile([128, TT], f32, name=f"r{c}", tag=f"r{c}") for c in range(NC)]
                for co in range(NC):
                    ps = psA.tile([128, TT], f32, name="ps_wo", tag="psA")
                    for c in range(NC):
                        nc.tensor.matmul(ps, lhsT=wo_sb[c][:, co * 128:(co + 1) * 128],
                                         rhs=oT[c], start=(c == 0), stop=False)
                    nc.tensor.matmul(ps, lhsT=bor[0:1, co * 128:(co + 1) * 128],
                                     rhs=ones_r, start=False, stop=True)
                    nc.vector.tensor_add(r_sb[co], ps, xt[co])
                    nc.sync.dma_start(out=rd[co * 128:(co + 1) * 128, t0:t0 + TT], in_=r_sb[co])
                m2row = act.tile([1, TT], bf16, name="m2row", tag="r_m2row", bufs=2)
                r2row = act.tile([1, TT], bf16, name="r2row", tag="r_r2row", bufs=2)
                ln_stats(r_sb, m2row, r2row, "L2")
                nc.sync.dma_start(out=m2d[0:1, t0:t0 + TT], in_=m2row)
                nc.sync.dma_start(out=r2d[0:1, t0:t0 + TT], in_=r2row)


            # =========================== PASS B1 (W1 + gelu) ==============
            w1_sb = []
            for i in range(NC * 4):
                c, qd = i // 4, i % 4
                t_ = wt.tile([128, DIM], bf16, name=f"w1_{i}", tag=f"wt{i}")
                nc.sync.dma_start(out=t_, in_=w1_e[c * 128:(c + 1) * 128,
                                                   qd * DIM:(qd + 1) * DIM])
                w1_sb.append(t_)
            for it in range(NT):
                t0 = it * TT
                rb1 = [act.tile([128, TT], f32, name=f"rb1_{c}", tag=f"r{c}") for c in range(NC)]
                for c in range(NC):
                    nc.sync.dma_start(out=rb1[c], in_=rd[c * 128:(c + 1) * 128, t0:t0 + TT])
                m2b = act.tile([1, TT], bf16, name="m2b", tag="r_m2row", bufs=2)
                r2b = act.tile([1, TT], bf16, name="r2b", tag="r_r2row", bufs=2)
                nc.sync.dma_start(out=m2b, in_=m2d[0:1, t0:t0 + TT])
                nc.sync.dma_start(out=r2b, in_=r2d[0:1, t0:t0 + TT])
                h2b = [act.tile([128, TT], bf16, name=f"h2b{c}", tag=f"h2_{c}", bufs=1)
                       for c in range(NC)]
                ln_apply(rb1, m2b, r2b, h2b, "B1")
                for hj in range(NH):
                    qd, sub = hj // 8, hj % 8
                    ps = psA.tile([128, TT], f32, name="ps_w1", tag="psA")
                    for c in range(NC):
                        nc.tensor.matmul(ps, lhsT=w1_sb[c * 4 + qd][:, sub * 128:(sub + 1) * 128],
                                         rhs=h2b[c], start=(c == 0), stop=(c == NC - 1))
                    g_sb = act.tile([128, TT], bf16, name="g_sb", tag="sq", bufs=2)
                    nc.scalar.activation(g_sb, ps, AF.Gelu, bias=b1c[:, hj:hj + 1])
                    nc.sync.dma_start(out=gd[hj * 128:(hj + 1) * 128, t0:t0 + TT], in_=g_sb)

            # =========================== PASS B2 (W2 + residual) ==========
            w2_sb = []
            for i in range(NH):
                t_ = wt.tile([128, DIM], bf16, name=f"w2_{i}", tag=f"wt{i}")
                nc.sync.dma_start(out=t_, in_=w2_e[i * 128:(i + 1) * 128, :])
                w2_sb.append(t_)
            GB_TAGS = [f"hb{i}" for i in range(8)] + [f"q{i}" for i in range(8)] + \
                      [f"k{i}" for i in range(8)] + [f"oT{i}" for i in range(8)]
            for it in range(NT):
                t0 = it * TT
                gb = [act.tile([128, TT], bf16, name=f"gb{hc}", tag=GB_TAGS[hc],
                               bufs=(2 if 8 <= hc < 24 else 1))
                      for hc in range(NH)]
                for hc in range(NH):
                    nc.sync.dma_start(out=gb[hc], in_=gd[hc * 128:(hc + 1) * 128, t0:t0 + TT])
                rb = [act.tile([128, TT], f32, name=f"rb{c}", tag=f"r{c}") for c in range(NC)]
                for c in range(NC):
                    nc.sync.dma_start(out=rb[c], in_=rd[c * 128:(c + 1) * 128, t0:t0 + TT])
                for co in range(NC):
                    ps = psA.tile([128, TT], f32, name="ps_w2", tag="psA")
                    for hc in range(NH):
                        nc.tensor.matmul(ps, lhsT=w2_sb[hc][:, co * 128:(co + 1) * 128],
                                         rhs=gb[hc], start=(hc == 0), stop=False)
                    nc.tensor.matmul(ps, lhsT=b2r[0:1, co * 128:(co + 1) * 128],
                                     rhs=ones_r, start=False, stop=True)
                    y_sb = act.tile([128, TT], f32, name="y_sb", tag=f"xt{co}", bufs=xin_bufs)
                    nc.vector.tensor_add(y_sb, ps, rb[co])
                    nc.sync.dma_start(out=yT_e[co * 128:(co + 1) * 128, t0:t0 + TT], in_=y_sb)

    _split_multi_waits(nc)
    return nc


# ---------------------------------------------------------------------------
# Host side
# ---------------------------------------------------------------------------
_CACHE = {}


def _bf(a):
    return np.ascontiguousarray(a).astype(ml_dtypes.bfloat16)


def prep_consts(g1, beta1, Wq, bq, Wk, bk, Wv, bv, Wo, bo, g2, beta2,
                W1, b1m, W2, b2m):
    Wq_e = (g1[:, None] * Wq) * SCALE
    bq_e = (beta1 @ Wq + bq) * SCALE
    Wk_e = g1[:, None] * Wk
    bk_e = beta1 @ Wk + bk
    Wv_e = g1[:, None] * Wv
    bv_e = beta1 @ Wv + bv
    bo_e = bv_e @ Wo + bo
    W1_e = g2[:, None] * W1
    b1_e = beta2 @ W1 + b1m
    # cols 0-7: bq chunks; cols 8-15: bk chunks
    bqk = np.concatenate([bq_e.reshape(8, 128).T, bk_e.reshape(8, 128).T], axis=1)
    sel = np.zeros((128, 256), np.float32)
    sel[0, 0:64] = 1.0       # even chunk: heads at rows 0 / 32
    sel[32, 64:128] = 1.0
    sel[64, 128 + 0:128 + 64] = 1.0   # odd chunk: rows 64 / 96
    sel[96, 128 + 64:128 + 128] = 1.0
    return {
        "wq": _bf(Wq_e), "wk": _bf(Wk_e), "wv": _bf(Wv_e), "wo": _bf(Wo),
        "w1": _bf(W1_e), "w2": _bf(W2),
        "bor": _bf(bo_e)[None, :], "b2r": _bf(b2m)[None, :],
        "bqk": np.ascontiguousarray(bqk.astype(np.float32)),
        "b1c": np.ascontiguousarray(b1_e.reshape(NH, 128).T.astype(np.float32)),
        "sel": _bf(sel),
    }


def window_order(x_b):
    # [4096, C] row-major spatial -> window-contiguous [4096, C]
    C = x_b.shape[-1]
    t = x_b.reshape(4, 16, 4, 16, C).transpose(0, 2, 1, 3, 4)
    return t.reshape(4096, C)


def window_unorder(y_b):
    C = y_b.shape[-1]
    t = y_b.reshape(4, 4, 16, 16, C).transpose(0, 2, 1, 3, 4)
    return t.reshape(4096, C)


def kernel(x, g1, beta1, Wq, bq, Wk, bk, Wv, bv, Wo, bo, g2, beta2,
           W1, b1m, W2, b2m, window_size, spatial_h, spatial_w):
    x = np.asarray(x, np.float32)
    args = [np.asarray(a, np.float32) for a in
            (g1, beta1, Wq, bq, Wk, bk, Wv, bv, Wo, bo, g2, beta2, W1, b1m, W2, b2m)]
    consts = prep_consts(*args)

    if "nc" not in _CACHE:
        _CACHE["nc"] = build_nc(NT=8)
    nc = _CACHE["nc"]

    B = x.shape[0]
    in_maps = []
    for c in range(B):
        xw = window_order(x[c])                       # [4096, C]
        m = {"xT": np.ascontiguousarray(xw.T)}        # [C, 4096] f32
        m.update(consts)
        in_maps.append(m)
    res = run_bass_kernel_spmd(nc, in_maps, core_ids=list(range(B)))
    out = np.empty_like(x)
    for c in range(B):
        yT = res.results[c]["yT"]                     # [C, 4096]
        out[c] = window_unorder(np.ascontiguousarray(yT.T))
    return out


# revision 17
# speedup vs baseline: 1.1969x; 1.0276x over previous
"""Trainium2 Bass kernel for nn_BlockDrop (Swin-style transformer block).

Reference math (per batch image):
  h = LN1(x); 16x16 windows of 256 tokens; 16-head attention (d=64) with
  separate Q/K/V/O linears; x += attn; h2 = LN2(x); x += W2@gelu(W1@h2).

Sharding: pure data parallel — batch image b -> core b (16 windows each,
no cross-core communication). Host performs window reordering,
transposition (feature-major) and weight folding; the NEFF does the rest.

In-kernel: activations feature-major [C, T]; bf16 matmuls, fp32 PSUM
accumulation, fp32 residual stream. LayerNorm stats via ones-matmuls;
rsqrt as exp(-0.5*ln(var+eps)) so pass A needs one ACT table set. The
LN mean offset and all biases are folded in as rank-1 K=1 matmuls.
Softmax: scores^T layout, no max-subtraction (inputs bounded), ones
column appended to V yields the softmax denominators inside the o-matmul;
1/s rows are broadcast via selector matmuls.

SBUF: one NEFF, three passes (attn+LN2 | W1+gelu | W2+residual) with
DRAM intermediates; weight slots and most activation slots are
tag-shared across passes.
"""
import numpy as np
import ml_dtypes

import concourse.bass as bass
import concourse.mybir as mybir
import concourse.tile as tile
from concourse.bass_utils import run_bass_kernel_spmd

f32 = mybir.dt.float32
f32r = mybir.dt.float32r
bf16 = mybir.dt.bfloat16
AF = mybir.ActivationFunctionType

DIM = 1024
HEADS = 16
HDIM = 64
HID = 4096
SCALE = HDIM ** -0.5
EPS = 1e-5
T = 4096          # tokens per core
TT = 512          # tokens per T-tile (2 windows)
NC = 8            # C chunks
NH = 32           # HID chunks
WS2 = 256         # tokens per window


def _split_multi_waits(nc):
    """This walrus rejects >1 sync-wait per instruction. Move extra waits
    onto same-engine NoOps inserted just before (engine queues are FIFO,
    so blocking the queue on each sem in turn is equivalent)."""
    n_split = 0
    for fn in nc.m.functions:
        for blk in fn.blocks:
            insts = blk.instructions
            new = []
            for inst in insts:
                si = inst.sync_info
                waits = list(si.on_wait) if si is not None else []
                if len(waits) > 1:
                    for w in waits[:-1]:
                        n_split += 1
                        new.append(mybir.InstNoOp(
                            name=f"{inst.name}-ws{n_split}",
                            engine=inst.engine, ins=[], outs=[],
                            sync_info=mybir.SyncInfo(on_wait=[w], on_update=[]),
                        ))
                    inst.sync_info = mybir.SyncInfo(
                        on_wait=[waits[-1]], on_update=list(si.on_update))
                new.append(inst)
            if len(new) != len(insts):
                blk.instructions[:] = new
    return n_split


def build_nc(NT=8, use_f32r=False, xin_bufs=1):
    nc = bass.Bass()

    xT_e = nc.declare_dram_parameter("xT", [DIM, T], f32, isOutput=False)
    wq_e = nc.declare_dram_parameter("wq", [DIM, DIM], bf16, isOutput=False)
    wk_e = nc.declare_dram_parameter("wk", [DIM, DIM], bf16, isOutput=False)
    wv_e = nc.declare_dram_parameter("wv", [DIM, DIM], bf16, isOutput=False)
    wo_e = nc.declare_dram_parameter("wo", [DIM, DIM], bf16, isOutput=False)
    w1_e = nc.declare_dram_parameter("w1", [DIM, HID], bf16, isOutput=False)
    w2_e = nc.declare_dram_parameter("w2", [HID, DIM], bf16, isOutput=False)
    bor_e = nc.declare_dram_parameter("bor", [1, DIM], bf16, isOutput=False)
    b2r_e = nc.declare_dram_parameter("b2r", [1, DIM], bf16, isOutput=False)
    bqk_e = nc.declare_dram_parameter("bqk", [128, 16], f32, isOutput=False)
    b1c_e = nc.declare_dram_parameter("b1c", [128, NH], f32, isOutput=False)
    sel_e = nc.declare_dram_parameter("sel", [128, 256], bf16, isOutput=False)
    yT_e = nc.declare_dram_parameter("yT", [DIM, T], f32, isOutput=True)

    rd = nc.dram_tensor("rd", [DIM, T], f32)        # post-attn residual
    m2d = nc.dram_tensor("m2d", [1, T], bf16)       # LN2 mean row
    r2d = nc.dram_tensor("r2d", [1, T], bf16)       # LN2 rstd row
    gd = nc.dram_tensor("gd", [HID, T], bf16)       # gelu(W1 h2 + b1)

    stat_dt = f32r if use_f32r else f32

    with tile.TileContext(nc) as tc:
        with (
            tc.tile_pool(name="wt", bufs=1) as wt,
            tc.tile_pool(name="cst", bufs=1) as cst,
            tc.tile_pool(name="act", bufs=1) as act,
            tc.tile_pool(name="psA", bufs=8, space="PSUM") as psA,
        ):
            # ---- constants ----
            bor = cst.tile([1, DIM], bf16)
            b2r = cst.tile([1, DIM], bf16)
            bqk = cst.tile([128, 16], f32)
            b1c = cst.tile([128, NH], f32)
            sel = cst.tile([128, 256], bf16)
            for dst, srcp in ((bor, bor_e), (b2r, b2r_e),
                              (bqk, bqk_e), (b1c, b1c_e), (sel, sel_e)):
                nc.sync.dma_start(out=dst, in_=srcp[:])
            ones_s = cst.tile([128, 1], f32)     # LN sum lhsT
            ones_q = cst.tile([128, 1], bf16)    # LN sumsq lhsT
            ones_r = cst.tile([1, TT], bf16)     # bias-fold rhs
            ones_b = cst.tile([1, 128], bf16)    # K=1 broadcast lhsT
            eps_t = cst.tile([1, 1], f32)
            nc.vector.memset(ones_s, 1.0)
            nc.vector.memset(ones_q, 1.0)
            nc.vector.memset(ones_r, 1.0)
            nc.vector.memset(ones_b, 1.0)
            nc.vector.memset(eps_t, EPS)

            # ---- pass-A weights in the 32 shared weight slots ----
            wq_sb, wk_sb, wv_sb, wo_sb = [], [], [], []
            for g, (lst, src) in enumerate((
                    (wq_sb, wq_e), (wk_sb, wk_e), (wv_sb, wv_e), (wo_sb, wo_e))):
                for c in range(NC):
                    t_ = wt.tile([128, DIM], bf16, name=f"wA{g}_{c}", tag=f"wt{g * 8 + c}")
                    nc.sync.dma_start(out=t_, in_=src[c * 128:(c + 1) * 128, :])
                    lst.append(t_)

            def ln_stats(src_tiles, mean_dst, rs_dst, tag):
                """mean/rstd (bf16 [1,TT] rows) of feature-major src tiles."""
                ps_s = psA.tile([1, TT], f32, name=f"ps_s{tag}", tag="psA")
                ps_q = psA.tile([1, TT], f32, name=f"ps_q{tag}", tag="psA")
                for c in range(NC):
                    sq = act.tile([128, TT], bf16, name=f"sq{tag}{c}", tag="sq", bufs=2)
                    nc.scalar.activation(sq, src_tiles[c], AF.Square)
                    nc.tensor.matmul(ps_s, lhsT=ones_s.bitcast(stat_dt),
                                     rhs=src_tiles[c].bitcast(stat_dt),
                                     start=(c == 0), stop=(c == NC - 1))
                    nc.tensor.matmul(ps_q, lhsT=ones_q, rhs=sq,
                                     start=(c == 0), stop=(c == NC - 1))
                meanf = act.tile([1, TT], f32, name=f"meanf{tag}", tag="r_meanf", bufs=1)
                exq = act.tile([1, TT], f32, name=f"exq{tag}", tag="r_exq", bufs=2)
                nc.scalar.activation(mean_dst, ps_s, AF.Copy, scale=1.0 / DIM)
                nc.scalar.activation(meanf, ps_s, AF.Copy, scale=1.0 / DIM)
                nc.scalar.activation(exq, ps_q, AF.Copy, scale=1.0 / DIM)
                m2 = act.tile([1, TT], f32, name=f"m2{tag}", tag="r_m2", bufs=1)
                nc.scalar.activation(m2, meanf, AF.Square)
                nc.vector.tensor_sub(exq, exq, m2)          # var (in place)
                lnv = act.tile([1, TT], f32, name=f"lnv{tag}", tag="r_lnv", bufs=1)
                nc.scalar.activation(lnv, exq, AF.Ln, bias=eps_t)
                nc.scalar.activation(rs_dst, lnv, AF.Exp, scale=-0.5)

            def ln_apply(src_tiles, mean_row, rs_row, dst_tiles, tag):
                ps_m = psA.tile([128, TT], f32, name=f"ps_m{tag}", tag="psA")
                nc.tensor.matmul(ps_m, lhsT=ones_b, rhs=mean_row, start=True, stop=True)
                ps_r = psA.tile([128, TT], f32, name=f"ps_r{tag}", tag="psA")
                nc.tensor.matmul(ps_r, lhsT=ones_b, rhs=rs_row, start=True, stop=True)
                for c in range(NC):
                    cen = act.tile([128, TT], f32, name=f"cen{tag}{c}", tag="cen", bufs=1)
                    nc.vector.tensor_sub(cen, src_tiles[c], ps_m)
                    nc.vector.tensor_mul(dst_tiles[c], cen, ps_r)

            # ======== PASS A0: LN1 stats for all tiles (pipelined) ========
            mean_all = cst.tile([1, T], bf16)
            rs1_all = cst.tile([1, T], bf16)
            for it in range(NT):
                t0 = it * TT
                xa = [act.tile([128, TT], f32, name=f"xa{c}", tag=f"xt{c}", bufs=xin_bufs)
                      for c in range(NC)]
                for c in range(NC):
                    nc.sync.dma_start(out=xa[c], in_=xT_e[c * 128:(c + 1) * 128, t0:t0 + TT])
                ln_stats(xa, mean_all[0:1, t0:t0 + TT], rs1_all[0:1, t0:t0 + TT], "A0")

            # =========================== PASS A ===========================
            for it in range(NT):
                t0 = it * TT
                xt = [act.tile([128, TT], f32, name=f"xt{c}", tag=f"xt{c}", bufs=xin_bufs)
                      for c in range(NC)]
                for c in range(NC):
                    nc.sync.dma_start(out=xt[c], in_=xT_e[c * 128:(c + 1) * 128, t0:t0 + TT])
                hb = [act.tile([128, TT], bf16, name=f"hb{c}", tag=f"hb{c}")
                      for c in range(NC)]
                ln_apply(xt, mean_all[0:1, t0:t0 + TT], rs1_all[0:1, t0:t0 + TT], hb, "L1")

                # ---- QKV ----
                q_sb = [act.tile([128, TT], bf16, name=f"q{c}", tag=f"q{c}", bufs=2) for c in range(NC)]
                k_sb = [act.tile([128, TT], bf16, name=f"k{c}", tag=f"k{c}", bufs=2) for c in range(NC)]
                for co in range(NC):
                    ps = psA.tile([128, TT], f32, name="ps_q", tag="psA")
                    for c in range(NC):
                        nc.tensor.matmul(ps, lhsT=wq_sb[c][:, co * 128:(co + 1) * 128],
                                         rhs=hb[c], start=(c == 0), stop=(c == NC - 1))
                    nc.any.tensor_scalar_add(q_sb[co], ps, bqk[:, co:co + 1])
                    ps = psA.tile([128, TT], f32, name="ps_k", tag="psA")
                    for c in range(NC):
                        nc.tensor.matmul(ps, lhsT=wk_sb[c][:, co * 128:(co + 1) * 128],
                                         rhs=hb[c], start=(c == 0), stop=(c == NC - 1))
                    nc.any.tensor_scalar_add(k_sb[co], ps, bqk[:, 8 + co:8 + co + 1])
                v_sb = [act.tile([128, HEADS, 65], bf16, name=f"v{tc_}", tag=f"v{tc_}")
                        for tc_ in range(4)]
                for tc_ in range(4):
                    for nh in range(2):
                        ps = psA.tile([128, TT], f32, name="ps_v", tag="psA")
                        for c in range(NC):
                            nc.tensor.matmul(ps, lhsT=hb[c][:, tc_ * 128:(tc_ + 1) * 128],
                                             rhs=wv_sb[c][:, nh * 512:(nh + 1) * 512],
                                             start=(c == 0), stop=(c == NC - 1))
                        nc.vector.tensor_copy(
                            v_sb[tc_][:, nh * 8:(nh + 1) * 8, 0:64],
                            ps.rearrange("p (h d) -> p h d", d=64))
                    nc.vector.memset(v_sb[tc_][:, :, 64:65], 1.0)

                # ---- attention ----
                sc = [act.tile([128, TT], bf16, name=f"sc{g}", tag=f"sc{g}", bufs=1) for g in range(4)]
                for g in range(4):
                    nc.vector.memset(sc[g], 1.0)
                oT = [act.tile([128, TT], bf16, name=f"oT{c}", tag=f"oT{c}") for c in range(NC)]
                for w in range(2):
                    ws = w * WS2
                    for h0 in range(0, HEADS, 4):
                        grp = range(h0, min(h0 + 4, HEADS))
                        ps_s_g, e_g, ps_o_g = {}, {}, {}
                        for h in grp:
                            ch, hh = h // 2, 64 * (h % 2)
                            ps_s = psA.tile([128, TT], f32, name="ps_sT", tag="psA")
                            nc.tensor.matmul(ps_s[:, 0:WS2],
                                             lhsT=k_sb[ch][hh:hh + 64, ws:ws + 128],
                                             rhs=q_sb[ch][hh:hh + 64, ws:ws + WS2],
                                             start=True, stop=False)
                            nc.tensor.matmul(ps_s[:, WS2:TT],
                                             lhsT=k_sb[ch][hh:hh + 64, ws + 128:ws + WS2],
                                             rhs=q_sb[ch][hh:hh + 64, ws:ws + WS2],
                                             start=False, stop=True)
                            ps_s_g[h] = ps_s
                        for h in grp:
                            e_sb = act.tile([128, TT], bf16, name="e_sb", tag="e", bufs=3)
                            nc.scalar.activation(e_sb, ps_s_g[h], AF.Exp)
                            e_g[h] = e_sb
                        for h in grp:
                            ps_o = psA.tile([65, WS2], f32, name="ps_o", tag="psA")
                            nc.tensor.matmul(ps_o, lhsT=v_sb[2 * w][:, h, :],
                                             rhs=e_g[h][:, 0:WS2], start=True, stop=False)
                            nc.tensor.matmul(ps_o, lhsT=v_sb[2 * w + 1][:, h, :],
                                             rhs=e_g[h][:, WS2:TT], start=False, stop=True)
                            ps_o_g[h] = ps_o
                        for h in grp:
                            ch, hh = h // 2, 64 * (h % 2)
                            nc.vector.tensor_copy(
                                sc[h // 4][32 * (h % 4):32 * (h % 4) + 1, ws:ws + WS2],
                                ps_o_g[h][64:65, :])
                            nc.any.tensor_copy(oT[ch][hh:hh + 64, ws:ws + WS2],
                                               ps_o_g[h][0:64, :])

                # ---- normalize (in place) + Wo + residual ----
                with nc.allow_low_precision(reason="1/s as bf16 matmul operand"):
                    for g in range(2):
                        nc.scalar.activation(sc[g], sc[g], AF.Ln)
                        nc.scalar.activation(sc[g], sc[g], AF.Exp, scale=-1.0)
                    for g in range(2, 4):
                        nc.vector.reciprocal(sc[g], sc[g])
                for j in range(NC):
                    ps_b = psA.tile([128, TT], f32, name="ps_rsb", tag="psA")
                    nc.tensor.matmul(ps_b, lhsT=sel[:, 128 * (j % 2):128 * (j % 2) + 128],
                                     rhs=sc[j // 2], start=True, stop=True)
                    nc.vector.tensor_mul(oT[j], oT[j], ps_b)
                r_sb = [act.tile([128, TT], f32, name=f"r{c}", tag=f"r{c}") for c in range(NC)]
                for co in range(NC):
                    ps = psA.tile([128, TT], f32, name="ps_wo", tag="psA")
                    for c in range(NC):
                        nc.tensor.matmul(ps, lhsT=wo_sb[c][:, co * 128:(co + 1) * 128],
                                         rhs=oT[c], start=(c == 0), stop=False)
                    nc.tensor.matmul(ps, lhsT=bor[0:1, co * 128:(co + 1) * 128],
                                     rhs=ones_r, start=False, stop=True)
                    nc.vector.tensor_add(r_sb[co], ps, xt[co])
                    nc.sync.dma_start(out=rd[co * 128:(co + 1) * 128, t0:t0 + TT], in_=r_sb[co])
                m2row = act.tile([1, TT], bf16, name="m2row", tag="r_m2row", bufs=2)
                r2row = act.tile([1, TT], bf16, name="r2row", tag="r_r2row", bufs=2)
                ln_stats(r_sb, m2row, r2row, "L2")
                nc.sync.dma_start(out=m2d[0:1, t0:t0 + TT], in_=m2row)
                nc.sync.dma_start(out=r2d[0:1, t0:t0 + TT], in_=r2row)


            # =========================== PASS B1 (W1 + gelu) ==============
            w1_sb = []
            for i in range(NC * 4):
                c, qd = i // 4, i % 4
                t_ = wt.tile([128, DIM], bf16, name=f"w1_{i}", tag=f"wt{i}")
                nc.sync.dma_start(out=t_, in_=w1_e[c * 128:(c + 1) * 128,
                                                   qd * DIM:(qd + 1) * DIM])
                w1_sb.append(t_)
            for it in range(NT):
                t0 = it * TT
                rb1 = [act.tile([128, TT], f32, name=f"rb1_{c}", tag=f"r{c}") for c in range(NC)]
                for c in range(NC):
                    nc.sync.dma_start(out=rb1[c], in_=rd[c * 128:(c + 1) * 128, t0:t0 + TT])
                m2b = act.tile([1, TT], bf16, name="m2b", tag="r_m2row", bufs=2)
                r2b = act.tile([1, TT], bf16, name="r2b", tag="r_r2row", bufs=2)
                nc.sync.dma_start(out=m2b, in_=m2d[0:1, t0:t0 + TT])
                nc.sync.dma_start(out=r2b, in_=r2d[0:1, t0:t0 + TT])
                h2b = [act.tile([128, TT], bf16, name=f"h2b{c}", tag=f"h2_{c}", bufs=1)
                       for c in range(NC)]
                ln_apply(rb1, m2b, r2b, h2b, "B1")
                for hj in range(NH):
                    qd, sub = hj // 8, hj % 8
                    ps = psA.tile([128, TT], f32, name="ps_w1", tag="psA")
                    for c in range(NC):
                        nc.tensor.matmul(ps, lhsT=w1_sb[c * 4 + qd][:, sub * 128:(sub + 1) * 128],
                                         rhs=h2b[c], start=(c == 0), stop=(c == NC - 1))
                    g_sb = act.tile([128, TT], bf16, name="g_sb", tag="sq", bufs=2)
                    nc.scalar.activation(g_sb, ps, AF.Gelu, bias=b1c[:, hj:hj + 1])
                    nc.sync.dma_start(out=gd[hj * 128:(hj + 1) * 128, t0:t0 + TT], in_=g_sb)

            # =========================== PASS B2 (W2 + residual) ==========
            w2_sb = []
            for i in range(NH):
                t_ = wt.tile([128, DIM], bf16, name=f"w2_{i}", tag=f"wt{i}")
                nc.sync.dma_start(out=t_, in_=w2_e[i * 128:(i + 1) * 128, :])
                w2_sb.append(t_)
            GB_TAGS = [f"hb{i}" for i in range(8)] + [f"q{i}" for i in range(8)] + \
                      [f"k{i}" for i in range(8)] + [f"oT{i}" for i in range(8)]
            for it in range(NT):
                t0 = it * TT
                gb = [act.tile([128, TT], bf16, name=f"gb{hc}", tag=GB_TAGS[hc],
                               bufs=(2 if 8 <= hc < 24 else 1))
                      for hc in range(NH)]
                for hc in range(NH):
                    nc.sync.dma_start(out=gb[hc], in_=gd[hc * 128:(hc + 1) * 128, t0:t0 + TT])
                rb = [act.tile([128, TT], f32, name=f"rb{c}", tag=f"r{c}") for c in range(NC)]
                for c in range(NC):
                    nc.sync.dma_start(out=rb[c], in_=rd[c * 128:(c + 1) * 128, t0:t0 + TT])
                for co in range(NC):
                    ps = psA.tile([128, TT], f32, name="ps_w2", tag="psA")
                    for hc in range(NH):
                        nc.tensor.matmul(ps, lhsT=w2_sb[hc][:, co * 128:(co + 1) * 128],
                                         rhs=gb[hc], start=(hc == 0), stop=False)
                    nc.tensor.matmul(ps, lhsT=b2r[0:1, co * 128:(co + 1) * 128],
                                     rhs=ones_r, start=False, stop=True)
                    y_sb = act.tile([128, TT], f32, name="y_sb", tag=f"xt{co}", bufs=xin_bufs)
                    nc.vector.tensor_add(y_sb, ps, rb[co])
                    nc.sync.dma_start(out=yT_e[co * 128:(co + 1) * 128, t0:t0 + TT], in_=y_sb)

    _split_multi_waits(nc)
    return nc


# ---------------------------------------------------------------------------
# Host side
# ---------------------------------------------------------------------------
_CACHE = {}


def _bf(a):
    return np.ascontiguousarray(a).astype(ml_dtypes.bfloat16)


def prep_consts(g1, beta1, Wq, bq, Wk, bk, Wv, bv, Wo, bo, g2, beta2,
                W1, b1m, W2, b2m):
    Wq_e = (g1[:, None] * Wq) * SCALE
    bq_e = (beta1 @ Wq + bq) * SCALE
    Wk_e = g1[:, None] * Wk
    bk_e = beta1 @ Wk + bk
    Wv_e = g1[:, None] * Wv
    bv_e = beta1 @ Wv + bv
    bo_e = bv_e @ Wo + bo
    W1_e = g2[:, None] * W1
    b1_e = beta2 @ W1 + b1m
    # cols 0-7: bq chunks; cols 8-15: bk chunks
    bqk = np.concatenate([bq_e.reshape(8, 128).T, bk_e.reshape(8, 128).T], axis=1)
    sel = np.zeros((128, 256), np.float32)
    sel[0, 0:64] = 1.0       # even chunk: heads at rows 0 / 32
    sel[32, 64:128] = 1.0
    sel[64, 128 + 0:128 + 64] = 1.0   # odd chunk: rows 64 / 96
    sel[96, 128 + 64:128 + 128] = 1.0
    return {
        "wq": _bf(Wq_e), "wk": _bf(Wk_e), "wv": _bf(Wv_e), "wo": _bf(Wo),
        "w1": _bf(W1_e), "w2": _bf(W2),
        "bor": _bf(bo_e)[None, :], "b2r": _bf(b2m)[None, :],
        "bqk": np.ascontiguousarray(bqk.astype(np.float32)),
        "b1c": np.ascontiguousarray(b1_e.reshape(NH, 128).T.astype(np.float32)),
        "sel": _bf(sel),
    }


def window_order(x_b):
    # [4096, C] row-major spatial -> window-contiguous [4096, C]
    C = x_b.shape[-1]
    t = x_b.reshape(4, 16, 4, 16, C).transpose(0, 2, 1, 3, 4)
    return t.reshape(4096, C)


def window_unorder(y_b):
    C = y_b.shape[-1]
    t = y_b.reshape(4, 4, 16, 16, C).transpose(0, 2, 1, 3, 4)
    return t.reshape(4096, C)


def kernel(x, g1, beta1, Wq, bq, Wk, bk, Wv, bv, Wo, bo, g2, beta2,
           W1, b1m, W2, b2m, window_size, spatial_h, spatial_w):
    x = np.asarray(x, np.float32)
    args = [np.asarray(a, np.float32) for a in
            (g1, beta1, Wq, bq, Wk, bk, Wv, bv, Wo, bo, g2, beta2, W1, b1m, W2, b2m)]
    consts = prep_consts(*args)

    if "nc" not in _CACHE:
        _CACHE["nc"] = build_nc(NT=8)
    nc = _CACHE["nc"]

    B = x.shape[0]
    in_maps = []
    for c in range(B):
        xw = window_order(x[c])                       # [4096, C]
        m = {"xT": np.ascontiguousarray(xw.T)}        # [C, 4096] f32
        m.update(consts)
        in_maps.append(m)
    res = run_bass_kernel_spmd(nc, in_maps, core_ids=list(range(B)))
    out = np.empty_like(x)
    for c in range(B):
        yT = res.results[c]["yT"]                     # [C, 4096]
        out[c] = window_unorder(np.ascontiguousarray(yT.T))
    return out
